# revision 21
# baseline (speedup 1.0000x reference)
"""Trainium2 Bass kernel for nn_Enigma_30502857736394 (dense transformer fwd + loss).

Sharding (8 cores, zero collectives):
  core c -> sequence s = c//2, token-half h = c%2.
  Both cores of a pair redundantly compute the full-sequence (T=512) trunk;
  the vocab head is token-split: each core does its own 256 tokens x full V.
  Each core PERMUTES tokens so its half sits at columns 0:255 (the trunk is
  token-permutation-equivariant; mask/embeddings carry the permutation), so
  one identical SPMD program serves all 8 cores.

Layouts: activations feature-partition [d, t] (f32r); weights f32r streamed
from DRAM; attention computed as S^T = K^T-layout matmul, exp without max
subtraction (scores bounded: weights ~N(0, 0.02)), causal mask multiplied in
as data, softmax denominator via a ones-column appended to V^T, output
divided by broadcast Z. Head in bf16 (DMA-bound there), logits fp32 out.
Loss assembled on host from device logits + device logZ.
"""
import numpy as np
import ml_dtypes

B, T, D, H, L, V = 4, 512, 512, 8, 8, 32000
HS, DF, EPS = 64, 4 * 512, 1e-5
TL = 256          # tokens per core in the head
NBLK = 2 * L      # 16 transformer blocks
VCH = 512         # head vocab chunk

_cache = {}
DEV_STATIC_W = False   # reuse block-0 weights (kill per-block weight DMAs)
DEV_FAST_LN = False    # skip Ln/Exp chain in LN (wrong numerics, timing only)


def _build_nc(lean=True):
    # lean=True: biases all zero + LN affine identity (verified by caller)
    import concourse.bacc as bacc
    import concourse.mybir as mybir
    import concourse.tile as tile
    import concourse.bass as bass

    DT = mybir.dt.float32r
    F32 = mybir.dt.float32
    BF = mybir.dt.bfloat16
    AF = mybir.ActivationFunctionType
    ALU = mybir.AluOpType

    # Route Ln and Exp to the combined natural_log_exp set (otherwise the
    # table-load pass picks separate sets and thrashes 5 loads per block).
    from concourse.hw_specs import get_activation_tables
    tabs = get_activation_tables("gen3")
    if AF.Exp in tabs["exp_and_others"]:
        assert AF.Exp in tabs["natural_log_exp_and_others"]
        assert AF.Ln in tabs["natural_log_exp_and_others"]
        tabs["exp_and_others"].discard(AF.Exp)
        tabs["natural_log"].discard(AF.Ln)

    nc = bacc.Bacc(trn_type="TRN2")

    din = lambda n, s, dt=DT: nc.dram_tensor(n, s, dt, kind="ExternalInput")
    xe = din("xe", (128, 4, T), F32)
    pe = din("pe", (128, 4, T), F32)
    msk = din("msk", (128, 4, T))
    qkw = din("qkw", (NBLK, 128, 4, 1024))
    vw = din("vw", (NBLK, 128, 4, 512))
    pw = din("pw", (NBLK, 128, 4, 512))
    f1w = din("f1w", (L, 128, 4, DF))
    f2w = din("f2w", (L, 128, 16, 512))
    pblk = din("pblk", (NBLK, 128, 40), F32)   # qkb|g1|b1|g2|b2|f1b columns
    prow = din("prow", (NBLK, 1, 1536))        # vbr|pbr|f2br rows (f32r)
    gf = din("gf", (128, 4), F32)
    bf = din("bf", (128, 4), F32)
    wout = din("wout", (128, 4, V), BF)
    bout = din("bout", (1, V), BF)
    logits = nc.dram_tensor("logits", (TL, V), F32, kind="ExternalOutput")
    logz = nc.dram_tensor("logz", (2, 128), F32, kind="ExternalOutput")

    from contextlib import ExitStack
    with ExitStack() as ctx:
        tc = ctx.enter_context(tile.TileContext(nc))
        pool = lambda n, b, **kw: ctx.enter_context(tc.tile_pool(name=n, bufs=b, **kw))
        constp = pool("const", 1)
        xp = pool("xp", 2)
        xnp = pool("xnp", 1)
        sqp = pool("sqp", 2)
        otp = pool("otp", 2)
        bp = pool("bp", 2)
        tp = pool("tp", 1)
        tpb = pool("tpb", 2)
        ps = pool("ps", 8, space="PSUM")
        trunk_ctx = ExitStack()
        tpool = lambda n, b, **kw: trunk_ctx.enter_context(tc.tile_pool(name=n, bufs=b, **kw))
        yp = tpool("yp", 1)
        vtp = tpool("vtp", 1)
        ep = tpool("ep", 8)
        opl = tpool("op", 1)
        hp = tpool("hp", 2)
        wq = tpool("wq", 2)
        wv = tpool("wv", 1)
        wpp = tpool("wpp", 1)
        w1p = tpool("w1p", 2)
        w2p = tpool("w2p", 2)
        if True:
            ones_f = constp.tile([128, 512], F32)
            nc.vector.memset(ones_f, 1.0)
            ones = constp.tile([128, 512], DT)
            nc.vector.tensor_copy(out=ones, in_=ones_f)
            cvals = [0.0, 1.0, -1.0, EPS, -0.5, float(HS) ** -0.5]
            cbuf = constp.tile([128, len(cvals)], F32)
            for i, v in enumerate(cvals):
                nc.vector.memset(cbuf[:, i:i + 1], v)
                nc.const_aps.aps[(F32, float(v))] = cbuf[:, i:i + 1]
            ones_bf = constp.tile([1, 128], BF)
            nc.vector.tensor_copy(out=ones_bf, in_=ones_f[0:1, 0:128])
            msk_sb = constp.tile([128, 4, T], DT)
            nc.sync.dma_start(out=msk_sb, in_=msk[:, :, :])

            # x = xe + pe  (residual stream, f32r, [d-part, d-tile, t])
            x = xp.tile([128, 4, T], DT, tag="x")
            xet = hp.tile([128, 4, T], F32, tag="h")
            pet = hp.tile([128, 4, T], F32, tag="h")
            nc.sync.dma_start(out=xet, in_=xe[:, :, :])
            nc.sync.dma_start(out=pet, in_=pe[:, :, :])
            nc.vector.tensor_tensor(out=x[:, :, :], in0=xet, in1=pet, op=ALU.add)

            def layer_norm(x_t, g_t, b_t, tcols):
                xn_t = xnp.tile([128, 4, T], DT, tag="xn")
                s1 = ps.tile([128, 512], F32, tag="ps")
                s2 = ps.tile([128, 512], F32, tag="ps")
                for k in range(4):
                    sq = sqp.tile([128, 512], DT, tag="sq")
                    nc.vector.tensor_tensor(out=sq[:, 0:tcols], in0=x_t[:, k, 0:tcols],
                                            in1=x_t[:, k, 0:tcols], op=ALU.mult)
                    nc.tensor.matmul(s1[0:1, 0:tcols], ones[:, 0:1], x_t[:, k, 0:tcols],
                                     start=(k == 0), stop=(k == 3))
                    nc.tensor.matmul(s2[0:1, 0:tcols], ones[:, 0:1], sq[:, 0:tcols],
                                     start=(k == 0), stop=(k == 3))
                mu = tpb.tile([1, 512], DT, tag="mu")
                m2 = tp.tile([1, 512], F32, tag="m2")
                var = tp.tile([1, 512], F32, tag="var")
                rstd = tpb.tile([1, 512], DT, tag="rstd")
                nc.vector.tensor_scalar(out=mu[0:1, 0:tcols], in0=s1[0:1, 0:tcols],
                                        scalar1=1.0 / D, scalar2=None, op0=ALU.mult)
                nc.vector.tensor_scalar(out=m2[0:1, 0:tcols], in0=s2[0:1, 0:tcols],
                                        scalar1=1.0 / D, scalar2=None, op0=ALU.mult)
                t1 = tp.tile([1, 512], F32, tag="t1")
                nc.vector.tensor_tensor(out=t1[0:1, 0:tcols], in0=mu[0:1, 0:tcols],
                                        in1=mu[0:1, 0:tcols], op=ALU.mult)
                nc.vector.tensor_tensor(out=var[0:1, 0:tcols], in0=m2[0:1, 0:tcols],
                                        in1=t1[0:1, 0:tcols], op=ALU.subtract)
                if DEV_FAST_LN:
                    nc.vector.tensor_copy(out=rstd[0:1, 0:tcols], in_=var[0:1, 0:tcols])
                else:
                    lv = tp.tile([1, 512], F32, tag="lv")
                    nc.scalar.activation(out=lv[0:1, 0:tcols], in_=var[0:1, 0:tcols],
                                         func=AF.Ln, bias=EPS)
                    nc.scalar.activation(out=rstd[0:1, 0:tcols], in_=lv[0:1, 0:tcols],
                                         func=AF.Exp, scale=-0.5)
                mub = ps.tile([128, 512], F32, tag="ps")
                rsb = ps.tile([128, 512], F32, tag="ps")
                nc.tensor.matmul(mub[:, 0:tcols], ones[0:1, 0:128], mu[0:1, 0:tcols],
                                 start=True, stop=True)
                nc.tensor.matmul(rsb[:, 0:tcols], ones[0:1, 0:128], rstd[0:1, 0:tcols],
                                 start=True, stop=True)
                for k in range(4):
                    if lean:
                        sc = sqp.tile([128, 512], F32, tag="sq")
                        nc.vector.tensor_tensor(out=sc[:, 0:tcols],
                                                in0=x_t[:, k, 0:tcols],
                                                in1=mub[:, 0:tcols], op=ALU.subtract)
                        nc.vector.tensor_tensor(out=xn_t[:, k, 0:tcols],
                                                in0=sc[:, 0:tcols],
                                                in1=rsb[:, 0:tcols], op=ALU.mult)
                    else:
                        sc = sqp.tile([128, 512], F32, tag="sq")
                        nc.vector.tensor_tensor(out=sc[:, 0:tcols], in0=x_t[:, k, 0:tcols],
                                                in1=mub[:, 0:tcols], op=ALU.subtract)
                        nc.vector.tensor_tensor(out=sc[:, 0:tcols], in0=sc[:, 0:tcols],
                                                in1=rsb[:, 0:tcols], op=ALU.mult)
                        nc.vector.tensor_scalar(out=xn_t[:, k, 0:tcols], in0=sc[:, 0:tcols],
                                                scalar1=g_t[:, k:k + 1], scalar2=b_t[:, k:k + 1],
                                                op0=ALU.mult, op1=ALU.add)
                return xn_t

            for bi in range(NBLK):
                step = bi // 2
                masked = (bi % 2 == 1)
                pbt_all = bp.tile([128, 40], F32, tag="pbt_all")
                nc.gpsimd.dma_start(out=pbt_all, in_=pblk[bi])
                prt = tp.tile([1, 1536], DT, tag="prt")
                nc.gpsimd.dma_start(out=prt, in_=prow[bi])
                qkbt = pbt_all[:, 0:8]
                g1t = pbt_all[:, 8:12]
                b1t = pbt_all[:, 12:16]
                g2t = pbt_all[:, 16:20]
                b2t = pbt_all[:, 20:24]
                f1bt = pbt_all[:, 24:40]
                vbt = prt[0:1, 0:512]
                pbt = prt[0:1, 512:1024]
                f2bt = prt[0:1, 1024:1536]

                xn = layer_norm(x, g1t, b1t, T)

                # ---- qkv (q,k feature-partition; v transposed) ----
                Y = yp.tile([128, 8, T], DT, tag="Y")
                for half in range(2):
                    wqs = wq.tile([128, 4, 512], DT, tag="wq")
                    nc.sync.dma_start(out=wqs, in_=qkw[0 if DEV_STATIC_W else bi, :, :, 512 * half:512 * half + 512])
                    for ft in range(4):
                        f = 4 * half + ft
                        o_ps = ps.tile([128, 512], F32, tag="ps")
                        for k in range(4):
                            nc.tensor.matmul(o_ps, wqs[:, k, 128 * ft:128 * ft + 128],
                                             xn[:, k, :], start=(k == 0), stop=(k == 3))
                        if lean:
                            if f % 2 == 0:
                                nc.vector.tensor_copy(out=Y[:, f, :], in_=o_ps)
                            else:
                                nc.scalar.activation(out=Y[:, f, :], in_=o_ps,
                                                     func=AF.Copy)
                        else:
                            nc.vector.tensor_scalar(out=Y[:, f, :], in0=o_ps,
                                                    scalar1=qkbt[:, f:f + 1],
                                                    scalar2=None, op0=ALU.add)
                # V^T with ones column per head (Z fusion)
                wvs = wv.tile([128, 4, 512], DT, tag="wv")
                nc.sync.dma_start(out=wvs, in_=vw[0 if DEV_STATIC_W else bi])
                Vt = vtp.tile([128, 4, 520], DT, tag="Vt")
                for tt in range(4):
                    v_ps = ps.tile([128, 512], F32, tag="ps")
                    if not lean:
                        nc.tensor.matmul(v_ps, ones[0:1, 0:128], vbt[0:1, :],
                                         start=True, stop=False)
                    for k in range(4):
                        nc.tensor.matmul(v_ps, xn[:, k, 128 * tt:128 * tt + 128],
                                         wvs[:, k, :], start=(lean and k == 0),
                                         stop=(k == 3))
                    vt_h = Vt[:, tt, :].rearrange("p (h c) -> p h c", c=65)
                    nc.vector.tensor_copy(
                        out=vt_h[:, :, 0:64],
                        in_=v_ps.rearrange("p (h c) -> p h c", c=64))
                    nc.vector.tensor_copy(
                        out=vt_h[:, :, 64:65],
                        in_=ones_f[:, 0:8].rearrange("p (h c) -> p h c", c=1))

                # ---- attention per head ----
                O_all = opl.tile([128, 4, T], DT, tag="O")
                O_odd = hp.tile([128, 4, T], DT, tag="h")
                for hh in range(8):
                    bq = 64 * (hh % 2)
                    ftq, ftk = hh // 2, 4 + hh // 2
                    Es = []
                    for i in range(4):
                        s_ps = ps.tile([128, 512], F32, tag="ps")
                        nc.tensor.matmul(s_ps,
                                         Y[bq:bq + 64, ftk, 128 * i:128 * i + 128],
                                         Y[bq:bq + 64, ftq, :], start=True, stop=True)
                        Ei = ep.tile([128, T], DT, tag="E", name=f"E{hh}_{i}")
                        nc.scalar.activation(out=Ei, in_=s_ps, func=AF.Exp,
                                             scale=float(HS) ** -0.5)
                        if masked:
                            meng = nc.vector if (hh + i) % 2 == 0 else nc.gpsimd
                            meng.tensor_tensor(out=Ei, in0=Ei,
                                               in1=msk_sb[:, i, :], op=ALU.mult)
                        Es.append(Ei)
                    oz = ps.tile([128, 512], F32, tag="ps")
                    for i in range(4):
                        nc.tensor.matmul(oz[0:65, :], Vt[:, i, 65 * hh:65 * hh + 65],
                                         Es[i], start=(i == 0), stop=(i == 3))
                    lnz = tpb.tile([128, 512], F32, tag="lnz")
                    nc.scalar.activation(out=lnz[64:65, :], in_=oz[64:65, :],
                                         func=AF.Ln)
                    rz = tpb.tile([128, 512], DT, tag="rz")
                    nc.scalar.activation(out=rz[64:65, :], in_=lnz[64:65, :],
                                         func=AF.Exp, scale=-1.0)
                    zb = ps.tile([128, 512], F32, tag="ps")
                    nc.tensor.matmul(zb, ones[64:65, 0:128], rz[64:65, :],
                                     start=True, stop=True)
                    zcp = otp.tile([128, 512], F32, tag="ot")
                    nc.vector.tensor_copy(out=zcp[0:64, :], in_=zb[0:64, :])
                    if hh % 2 == 0:
                        nc.vector.tensor_tensor(out=O_all[0:64, hh // 2, :],
                                                in0=oz[0:64, :], in1=zcp[0:64, :],
                                                op=ALU.mult)
                    else:
                        nc.vector.tensor_tensor(out=O_odd[0:64, hh // 2, :],
                                                in0=oz[0:64, :],
                                                in1=zcp[0:64, :], op=ALU.mult)
                if True:
                    nc.sync.dma_start(out=O_all[64:128, :, :], in_=O_odd[0:64, :, :])

                # ---- proj + residual ----
                wps = wpp.tile([128, 4, 512], DT, tag="wp")
                nc.sync.dma_start(out=wps, in_=pw[0 if DEV_STATIC_W else bi])
                x2 = xp.tile([128, 4, T], DT, tag="x")
                for f in range(4):
                    p_ps = ps.tile([128, 512], F32, tag="ps")
                    if not lean:
                        nc.tensor.matmul(p_ps, pbt[0:1, 128 * f:128 * f + 128],
                                         ones[0:1, :], start=True, stop=False)
                    for k in range(4):
                        nc.tensor.matmul(p_ps, wps[:, k, 128 * f:128 * f + 128],
                                         O_all[:, k, :], start=(lean and k == 0),
                                         stop=(k == 3))
                    nc.vector.tensor_tensor(out=x2[:, f, :], in0=x[:, f, :],
                                            in1=p_ps, op=ALU.add)
                x = x2

                # ---- FF ----
                xn2 = layer_norm(x, g2t, b2t, T)
                f2ps = [ps.tile([128, 512], F32, tag="ps", name=f"f2ps{i}")
                        for i in range(4)]
                if not lean:
                    for f in range(4):
                        nc.tensor.matmul(f2ps[f], f2bt[0:1, 128 * f:128 * f + 128],
                                         ones[0:1, :], start=True, stop=False)
                for q in range(4):
                    w1s = w1p.tile([128, 4, 512], DT, tag="w1")
                    nc.gpsimd.dma_start(out=w1s, in_=f1w[0 if DEV_STATIC_W else step, :, :, 512 * q:512 * q + 512])
                    hq = hp.tile([128, 4, T], DT, tag="h")
                    for ft in range(4):
                        h_ps = ps.tile([128, 512], F32, tag="ps")
                        for k in range(4):
                            nc.tensor.matmul(h_ps, w1s[:, k, 128 * ft:128 * ft + 128],
                                             xn2[:, k, :], start=(k == 0), stop=(k == 3))
                        fb = 4 * q + ft
                        nc.scalar.activation(out=hq[:, ft, :], in_=h_ps, func=AF.Gelu,
                                             bias=0.0 if lean else f1bt[:, fb:fb + 1])
                    w2s = w2p.tile([128, 4, 512], DT, tag="w2")
                    nc.gpsimd.dma_start(out=w2s, in_=f2w[0 if DEV_STATIC_W else step, :, 4 * q:4 * q + 4, :])
                    for f in range(4):
                        for kk in range(4):
                            nc.tensor.matmul(f2ps[f], w2s[:, kk, 128 * f:128 * f + 128],
                                             hq[:, kk, :],
                                             start=(lean and q == 0 and kk == 0),
                                             stop=(q == 3 and kk == 3))
                x3 = xp.tile([128, 4, T], DT, tag="x")
                for f in range(4):
                    nc.vector.tensor_tensor(out=x3[:, f, :], in0=x[:, f, :],
                                            in1=f2ps[f], op=ALU.add)
                x = x3

            # ---- head: lnf -> bf16 logits (token-partition) + logZ ----
            trunk_ctx.close()
            wop = ctx.enter_context(tc.tile_pool(name="wop", bufs=3))
            lsp = ctx.enter_context(tc.tile_pool(name="lsp", bufs=2))
            gft = bp.tile([128, 4], F32, tag="g1t")
            bft = bp.tile([128, 4], F32, tag="b1t")
            nc.sync.dma_start(out=gft, in_=gf[:, :])
            nc.sync.dma_start(out=bft, in_=bf[:, :])
            xnf = layer_norm(x, gft, bft, TL)
            xnb = constp.tile([128, 4, TL], BF)
            for k in range(4):
                nc.vector.tensor_copy(out=xnb[:, k, :], in_=xnf[:, k, 0:TL])

            bo = ctx.enter_context(tc.tile_pool(name="bop", bufs=1))
            bot = bo.tile([1, V], BF)
            nc.sync.dma_start(out=bot, in_=bout[0:1, :])
            GRP = 4                      # vocab chunks per DMA batch
            VB = GRP * VCH               # 2048
            ngrp = (V + VB - 1) // VB    # 16 (last partial: 32000 = 15*2048 + 1280)
            for tt in range(2):
                zacc = tpb.tile([128, 64], F32, tag="zacc")
                ci = 0
                for gi in range(ngrp):
                    g0 = gi * VB
                    gn = min(VB, V - g0)
                    wos = wop.tile([128, 4, VB], BF, tag="wo")
                    eng = nc.scalar if gi % 2 == 0 else nc.sync
                    eng.dma_start(out=wos[:, :, 0:gn], in_=wout[:, :, g0:g0 + gn])
                    lsb = lsp.tile([128, VB], F32, tag="lsb")
                    nsub = (gn + VCH - 1) // VCH
                    for si in range(nsub):
                        v0 = g0 + si * VCH
                        vn = min(VCH, V - v0)
                        l_ps = ps.tile([128, 512], F32, tag="ps")
                        if not lean:
                            nc.tensor.matmul(l_ps[:, 0:vn], ones_bf[0:1, :],
                                             bot[0:1, v0:v0 + vn], start=True, stop=False)
                        for k in range(4):
                            nc.tensor.matmul(l_ps[:, 0:vn],
                                             xnb[:, k, 128 * tt:128 * tt + 128],
                                             wos[:, k, si * VCH:si * VCH + vn],
                                             start=(lean and k == 0), stop=(k == 3))
                        nc.vector.tensor_copy(out=lsb[:, si * VCH:si * VCH + vn],
                                              in_=l_ps[:, 0:vn])
                        esc = lsp.tile([128, 512], F32, tag="esc")
                        nc.scalar.activation(out=esc[:, 0:vn], in_=l_ps[:, 0:vn],
                                             func=AF.Exp, accum_out=zacc[:, ci:ci + 1])
                        ci += 1
                    nc.gpsimd.dma_start(out=logits[128 * tt:128 * tt + 128, g0:g0 + gn],
                                        in_=lsb[:, 0:gn])
                z = tp.tile([128, 1], F32, tag="z")
                nc.vector.reduce_sum(z, zacc[:, 0:ci], axis=mybir.AxisListType.X)
                lz = tp.tile([128, 1], F32, tag="lz")
                nc.scalar.activation(out=lz, in_=z, func=AF.Ln)
                nc.sync.dma_start(out=logz[tt:tt + 1, :], in_=lz[:, 0:1])

    nc.compile()
    return nc


def _prep_inputs(inputs):
    """Host-side sharding: per-core in_maps (identical program, per-core data)."""
    f32 = np.float32
    bf16 = ml_dtypes.bfloat16
    idx = np.asarray(inputs["idx"])
    tok_emb = np.asarray(inputs["tok_emb"], f32)
    pos_emb = np.asarray(inputs["pos_emb"], f32)

    def dtile(a2d):
        # [D, cols] -> [128, 4, cols]  (d = k*128 + p)
        return np.ascontiguousarray(a2d.reshape(4, 128, -1).transpose(1, 0, 2))

    # weights, shared across cores
    shared = {}
    qkw = np.zeros((NBLK, 128, 4, 1024), f32)
    vw = np.zeros((NBLK, 128, 4, 512), f32)
    pw = np.zeros((NBLK, 128, 4, 512), f32)
    pblk = np.zeros((NBLK, 128, 40), f32)
    prow = np.zeros((NBLK, 1, 1536), f32)
    pack_ln1 = lambda a: np.asarray(a, f32).reshape(4, 128).T
    for l in range(L):
        for blk in range(2):
            bi = 2 * l + blk
            w = np.asarray(inputs["un_qkv_w" if blk == 0 else "m_qkv_w"][l], f32)
            b = np.asarray(inputs["un_qkv_b" if blk == 0 else "m_qkv_b"][l], f32)
            wq_ = np.concatenate([w[0], w[1]], axis=1)       # [D, 1024] q|k
            qkw[bi] = dtile(wq_)
            vw[bi] = dtile(w[2])
            pw_ = np.asarray(inputs["un_proj_w" if blk == 0 else "m_proj_w"][l], f32)
            pb_ = np.asarray(inputs["un_proj_b" if blk == 0 else "m_proj_b"][l], f32)
            pw[bi] = dtile(pw_)
            pblk[bi, :, 0:8] = np.concatenate([b[0], b[1]]).reshape(8, 128).T
            pblk[bi, :, 8:12] = pack_ln1(inputs["ln1_g"][l])
            pblk[bi, :, 12:16] = pack_ln1(inputs["ln1_b"][l])
            pblk[bi, :, 16:20] = pack_ln1(inputs["ln2_g"][l])
            pblk[bi, :, 20:24] = pack_ln1(inputs["ln2_b"][l])
            pblk[bi, :, 24:40] = np.asarray(inputs["ff_b1"][l], f32).reshape(16, 128).T
            prow[bi, 0, 0:512] = b[2]
            prow[bi, 0, 512:1024] = pb_
            prow[bi, 0, 1024:1536] = np.asarray(inputs["ff_b2"][l], f32)
    f1w = np.stack([dtile(np.asarray(inputs["ff_w1"][l], f32)) for l in range(L)])
    f2w = np.stack([np.ascontiguousarray(
        np.asarray(inputs["ff_w2"][l], f32).reshape(16, 128, 512).transpose(1, 0, 2))
        for l in range(L)])
    shared.update(qkw=qkw, vw=vw, pw=pw, f1w=f1w, f2w=f2w,
                  pblk=pblk, prow=prow,
                  gf=np.asarray(inputs["lnf_g"], f32).reshape(4, 128).T,
                  bf=np.asarray(inputs["lnf_b"], f32).reshape(4, 128).T,
                  wout=dtile(np.asarray(inputs["w_out"], f32)).astype(bf16),
                  bout=np.asarray(inputs["b_out"], f32).reshape(1, V).astype(bf16))

    in_maps = []
    for c in range(8):
        s, h = c // 2, c % 2
        perm = np.r_[h * TL:(h + 1) * TL, (1 - h) * TL:(2 - h) * TL]
        xe = dtile(tok_emb[idx[s, perm]].T)
        pe = dtile(pos_emb[perm].T)
        gl = perm  # glob token index per permuted position
        mask2d = (gl[:, None] <= gl[None, :]).astype(f32)      # [tk', tq'] allow
        mk = np.ascontiguousarray(mask2d.reshape(4, 128, T).transpose(1, 0, 2))
        m = dict(shared)
        m.update(xe=xe, pe=pe, msk=mk)
        in_maps.append(m)
    return in_maps


def kernel(**inputs):
    from concourse.bass_utils import run_bass_kernel_spmd

    lean = all(
        not np.any(np.asarray(inputs[k]))
        for k in ("un_qkv_b", "un_proj_b", "m_qkv_b", "m_proj_b",
                  "ff_b1", "ff_b2", "ln1_b", "ln2_b", "lnf_b", "b_out")
    ) and all(
        np.all(np.asarray(inputs[k]) == 1.0)
        for k in ("ln1_g", "ln2_g", "lnf_g")
    )
    key = ("nc", lean)
    if key not in _cache:
        _cache[key] = _build_nc(lean=lean)
        _cache["nc"] = _cache[key]
    nc = _cache[key]
    in_maps = _prep_inputs(inputs)
    res = run_bass_kernel_spmd(nc, in_maps, core_ids=list(range(8)))

    idx = np.asarray(inputs["idx"])
    targets = np.asarray(inputs["targets"])
    logits = np.zeros((B, T, V), np.float32)
    logz = np.zeros((B, T), np.float32)
    for c in range(8):
        s, h = c // 2, c % 2
        sl = slice(h * TL, (h + 1) * TL)
        logits[s, sl] = res.results[c]["logits"]
        logz[s, sl] = res.results[c]["logz"].reshape(TL)
    gathered = np.take_along_axis(
        logits.reshape(B * T, V),
        targets.reshape(B * T, 1).astype(np.int64), 1)[:, 0]
    loss = np.float32(np.mean(logz.reshape(B * T) - gathered))
    return logits, loss


# revision 23
# speedup vs baseline: 1.0621x; 1.0621x over previous
"""Trainium2 Bass kernel for nn_Enigma_30502857736394 (dense transformer fwd + loss).

Sharding (8 cores, zero collectives):
  core c -> sequence s = c//2, token-half h = c%2.
  Both cores of a pair redundantly compute the full-sequence (T=512) trunk;
  the vocab head is token-split: each core does its own 256 tokens x full V.
  Each core PERMUTES tokens so its half sits at columns 0:255 (the trunk is
  token-permutation-equivariant; mask/embeddings carry the permutation), so
  one identical SPMD program serves all 8 cores.

Layouts: activations feature-partition [d, t] (f32r); weights f32r streamed
from DRAM; attention computed as S^T = K^T-layout matmul, exp without max
subtraction (scores bounded: weights ~N(0, 0.02)), causal mask multiplied in
as data, softmax denominator via a ones-column appended to V^T, output
divided by broadcast Z. Head in bf16 (DMA-bound there), logits fp32 out.
Loss assembled on host from device logits + device logZ.
"""
import numpy as np
import ml_dtypes

B, T, D, H, L, V = 4, 512, 512, 8, 8, 32000
HS, DF, EPS = 64, 4 * 512, 1e-5
TL = 256          # tokens per core in the head
NBLK = 2 * L      # 16 transformer blocks
VCH = 512         # head vocab chunk

_cache = {}
DEV_STATIC_W = False   # reuse block-0 weights (kill per-block weight DMAs)
DEV_FAST_LN = False    # skip Ln/Exp chain in LN (wrong numerics, timing only)


def _build_nc(lean=True):
    # lean=True: biases all zero + LN affine identity (verified by caller)
    import concourse.bacc as bacc
    import concourse.mybir as mybir
    import concourse.tile as tile
    import concourse.bass as bass

    DT = mybir.dt.float32r
    F32 = mybir.dt.float32
    BF = mybir.dt.bfloat16
    AF = mybir.ActivationFunctionType
    ALU = mybir.AluOpType

    # Route Ln and Exp to the combined natural_log_exp set (otherwise the
    # table-load pass picks separate sets and thrashes 5 loads per block).
    from concourse.hw_specs import get_activation_tables
    tabs = get_activation_tables("gen3")
    if AF.Exp in tabs["exp_and_others"]:
        assert AF.Exp in tabs["natural_log_exp_and_others"]
        assert AF.Ln in tabs["natural_log_exp_and_others"]
        tabs["exp_and_others"].discard(AF.Exp)
        tabs["natural_log"].discard(AF.Ln)

    nc = bacc.Bacc(trn_type="TRN2")

    din = lambda n, s, dt=DT: nc.dram_tensor(n, s, dt, kind="ExternalInput")
    xe = din("xe", (128, 4, T), F32)
    pe = din("pe", (128, 4, T), F32)
    msk = din("msk", (128, 4, T))
    qkw = din("qkw", (NBLK, 128, 4, 1024))
    vw = din("vw", (NBLK, 128, 4, 512))
    pw = din("pw", (NBLK, 128, 4, 512))
    f1w = din("f1w", (L, 128, 4, DF))
    f2w = din("f2w", (L, 128, 16, 512))
    pblk = din("pblk", (NBLK, 128, 40), F32)   # qkb|g1|b1|g2|b2|f1b columns
    prow = din("prow", (NBLK, 1, 1536))        # vbr|pbr|f2br rows (f32r)
    gf = din("gf", (128, 4), F32)
    bf = din("bf", (128, 4), F32)
    wout = din("wout", (128, 4, V), BF)
    bout = din("bout", (1, V), BF)
    logits = nc.dram_tensor("logits", (TL, V), F32, kind="ExternalOutput")
    logz = nc.dram_tensor("logz", (2, 128), F32, kind="ExternalOutput")

    from contextlib import ExitStack
    with ExitStack() as ctx:
        tc = ctx.enter_context(tile.TileContext(nc))
        pool = lambda n, b, **kw: ctx.enter_context(tc.tile_pool(name=n, bufs=b, **kw))
        constp = pool("const", 1)
        xp = pool("xp", 2)
        xnp = pool("xnp", 1)
        sqp = pool("sqp", 2)
        otp = pool("otp", 2)
        bp = pool("bp", 2)
        tp = pool("tp", 1)
        tpb = pool("tpb", 2)
        ps = pool("ps", 4, space="PSUM")
        ps4 = pool("ps4", 1, space="PSUM")
        trunk_ctx = ExitStack()
        tpool = lambda n, b, **kw: trunk_ctx.enter_context(tc.tile_pool(name=n, bufs=b, **kw))
        yp = tpool("yp", 1)
        vtp = tpool("vtp", 1)
        ep = tpool("ep", 2)
        opl = tpool("op", 1)
        hp = tpool("hp", 2)
        wq = tpool("wq", 2)
        wv = tpool("wv", 1)
        wpp = tpool("wpp", 1)
        w1p = tpool("w1p", 2)
        w2p = tpool("w2p", 2)
        if True:
            ones_f = constp.tile([128, 512], F32)
            nc.vector.memset(ones_f, 1.0)
            ones = constp.tile([128, 512], DT)
            nc.vector.tensor_copy(out=ones, in_=ones_f)
            cvals = [0.0, 1.0, -1.0, EPS, -0.5, float(HS) ** -0.5]
            cbuf = constp.tile([128, len(cvals)], F32)
            for i, v in enumerate(cvals):
                nc.vector.memset(cbuf[:, i:i + 1], v)
                nc.const_aps.aps[(F32, float(v))] = cbuf[:, i:i + 1]
            ones_bf = constp.tile([1, 128], BF)
            nc.vector.tensor_copy(out=ones_bf, in_=ones_f[0:1, 0:128])
            msk_sb = constp.tile([128, 4, T], DT)
            nc.sync.dma_start(out=msk_sb, in_=msk[:, :, :])

            # x = xe + pe  (residual stream, f32r, [d-part, d-tile, t])
            x = xp.tile([128, 4, T], DT, tag="x")
            xet = hp.tile([128, 4, T], F32, tag="h")
            pet = hp.tile([128, 4, T], F32, tag="h")
            nc.sync.dma_start(out=xet, in_=xe[:, :, :])
            nc.sync.dma_start(out=pet, in_=pe[:, :, :])
            nc.vector.tensor_tensor(out=x[:, :, :], in0=xet, in1=pet, op=ALU.add)

            def layer_norm(x_t, g_t, b_t, tcols):
                xn_t = xnp.tile([128, 4, T], DT, tag="xn")
                s1 = ps.tile([128, 512], F32, tag="ps")
                s2 = ps.tile([128, 512], F32, tag="ps")
                for k in range(4):
                    sq = sqp.tile([128, 512], DT, tag="sq")
                    nc.vector.tensor_tensor(out=sq[:, 0:tcols], in0=x_t[:, k, 0:tcols],
                                            in1=x_t[:, k, 0:tcols], op=ALU.mult)
                    nc.tensor.matmul(s1[0:1, 0:tcols], ones[:, 0:1], x_t[:, k, 0:tcols],
                                     start=(k == 0), stop=(k == 3))
                    nc.tensor.matmul(s2[0:1, 0:tcols], ones[:, 0:1], sq[:, 0:tcols],
                                     start=(k == 0), stop=(k == 3))
                mu = tpb.tile([1, 512], DT, tag="mu")
                m2 = tp.tile([1, 512], F32, tag="m2")
                var = tp.tile([1, 512], F32, tag="var")
                rstd = tpb.tile([1, 512], DT, tag="rstd")
                nc.vector.tensor_scalar(out=mu[0:1, 0:tcols], in0=s1[0:1, 0:tcols],
                                        scalar1=1.0 / D, scalar2=None, op0=ALU.mult)
                nc.vector.tensor_scalar(out=m2[0:1, 0:tcols], in0=s2[0:1, 0:tcols],
                                        scalar1=1.0 / D, scalar2=None, op0=ALU.mult)
                t1 = tp.tile([1, 512], F32, tag="t1")
                nc.vector.tensor_tensor(out=t1[0:1, 0:tcols], in0=mu[0:1, 0:tcols],
                                        in1=mu[0:1, 0:tcols], op=ALU.mult)
                nc.vector.tensor_tensor(out=var[0:1, 0:tcols], in0=m2[0:1, 0:tcols],
                                        in1=t1[0:1, 0:tcols], op=ALU.subtract)
                if DEV_FAST_LN:
                    nc.vector.tensor_copy(out=rstd[0:1, 0:tcols], in_=var[0:1, 0:tcols])
                else:
                    lv = tp.tile([1, 512], F32, tag="lv")
                    nc.scalar.activation(out=lv[0:1, 0:tcols], in_=var[0:1, 0:tcols],
                                         func=AF.Ln, bias=EPS)
                    nc.scalar.activation(out=rstd[0:1, 0:tcols], in_=lv[0:1, 0:tcols],
                                         func=AF.Exp, scale=-0.5)
                mub = ps.tile([128, 512], F32, tag="ps")
                rsb = ps.tile([128, 512], F32, tag="ps")
                nc.tensor.matmul(mub[:, 0:tcols], ones[0:1, 0:128], mu[0:1, 0:tcols],
                                 start=True, stop=True)
                nc.tensor.matmul(rsb[:, 0:tcols], ones[0:1, 0:128], rstd[0:1, 0:tcols],
                                 start=True, stop=True)
                for k in range(4):
                    if lean:
                        sc = sqp.tile([128, 512], F32, tag="sq")
                        nc.vector.tensor_tensor(out=sc[:, 0:tcols],
                                                in0=x_t[:, k, 0:tcols],
                                                in1=mub[:, 0:tcols], op=ALU.subtract)
                        nc.vector.tensor_tensor(out=xn_t[:, k, 0:tcols],
                                                in0=sc[:, 0:tcols],
                                                in1=rsb[:, 0:tcols], op=ALU.mult)
                    else:
                        sc = sqp.tile([128, 512], F32, tag="sq")
                        nc.vector.tensor_tensor(out=sc[:, 0:tcols], in0=x_t[:, k, 0:tcols],
                                                in1=mub[:, 0:tcols], op=ALU.subtract)
                        nc.vector.tensor_tensor(out=sc[:, 0:tcols], in0=sc[:, 0:tcols],
                                                in1=rsb[:, 0:tcols], op=ALU.mult)
                        nc.vector.tensor_scalar(out=xn_t[:, k, 0:tcols], in0=sc[:, 0:tcols],
                                                scalar1=g_t[:, k:k + 1], scalar2=b_t[:, k:k + 1],
                                                op0=ALU.mult, op1=ALU.add)
                return xn_t

            for bi in range(NBLK):
                step = bi // 2
                masked = (bi % 2 == 1)
                pbt_all = bp.tile([128, 40], F32, tag="pbt_all")
                nc.gpsimd.dma_start(out=pbt_all, in_=pblk[bi])
                prt = tp.tile([1, 1536], DT, tag="prt")
                nc.gpsimd.dma_start(out=prt, in_=prow[bi])
                qkbt = pbt_all[:, 0:8]
                g1t = pbt_all[:, 8:12]
                b1t = pbt_all[:, 12:16]
                g2t = pbt_all[:, 16:20]
                b2t = pbt_all[:, 20:24]
                f1bt = pbt_all[:, 24:40]
                vbt = prt[0:1, 0:512]
                pbt = prt[0:1, 512:1024]
                f2bt = prt[0:1, 1024:1536]

                xn = layer_norm(x, g1t, b1t, T)

                # ---- qkv (q,k feature-partition; v transposed) ----
                Y = yp.tile([128, 8, T], DT, tag="Y")
                for half in range(2):
                    wqs = wq.tile([128, 4, 512], DT, tag="wq")
                    nc.sync.dma_start(out=wqs, in_=qkw[0 if DEV_STATIC_W else bi, :, :, 512 * half:512 * half + 512])
                    for ft in range(4):
                        f = 4 * half + ft
                        o_ps = ps.tile([128, 512], F32, tag="ps")
                        for k in range(4):
                            nc.tensor.matmul(o_ps, wqs[:, k, 128 * ft:128 * ft + 128],
                                             xn[:, k, :], start=(k == 0), stop=(k == 3))
                        if lean:
                            if f % 2 == 0:
                                nc.vector.tensor_copy(out=Y[:, f, :], in_=o_ps)
                            else:
                                nc.scalar.activation(out=Y[:, f, :], in_=o_ps,
                                                     func=AF.Copy)
                        else:
                            nc.vector.tensor_scalar(out=Y[:, f, :], in0=o_ps,
                                                    scalar1=qkbt[:, f:f + 1],
                                                    scalar2=None, op0=ALU.add)
                # V^T with ones column per head (Z fusion)
                wvs = wv.tile([128, 4, 512], DT, tag="wv")
                nc.sync.dma_start(out=wvs, in_=vw[0 if DEV_STATIC_W else bi])
                Vt = vtp.tile([128, 4, 520], DT, tag="Vt")
                for tt in range(4):
                    v_ps = ps.tile([128, 512], F32, tag="ps")
                    if not lean:
                        nc.tensor.matmul(v_ps, ones[0:1, 0:128], vbt[0:1, :],
                                         start=True, stop=False)
                    for k in range(4):
                        nc.tensor.matmul(v_ps, xn[:, k, 128 * tt:128 * tt + 128],
                                         wvs[:, k, :], start=(lean and k == 0),
                                         stop=(k == 3))
                    vt_h = Vt[:, tt, :].rearrange("p (h c) -> p h c", c=65)
                    nc.vector.tensor_copy(
                        out=vt_h[:, :, 0:64],
                        in_=v_ps.rearrange("p (h c) -> p h c", c=64))
                    nc.vector.tensor_copy(
                        out=vt_h[:, :, 64:65],
                        in_=ones_f[:, 0:8].rearrange("p (h c) -> p h c", c=1))

                # ---- attention per head ----
                O_all = opl.tile([128, 4, T], DT, tag="O")
                O_odd = hp.tile([128, 4, T], DT, tag="h")
                for hh in range(8):
                    bq = 64 * (hh % 2)
                    ftq, ftk = hh // 2, 4 + hh // 2
                    s_big = ps4.tile([128, 4, 512], F32, tag="ps4", name=f"S{hh}")
                    for i in range(4):
                        nc.tensor.matmul(s_big[:, i, :],
                                         Y[bq:bq + 64, ftk, 128 * i:128 * i + 128],
                                         Y[bq:bq + 64, ftq, :], start=True, stop=True)
                    Eb = ep.tile([128, 4, T], DT, tag="E", name=f"E{hh}")
                    nc.scalar.activation(out=Eb[:, :, :], in_=s_big[:, :, :],
                                         func=AF.Exp, scale=float(HS) ** -0.5)
                    if masked:
                        meng = nc.vector if hh % 2 == 0 else nc.gpsimd
                        meng.tensor_tensor(out=Eb[:, :, :], in0=Eb[:, :, :],
                                           in1=msk_sb[:, :, :], op=ALU.mult)
                    oz = ps.tile([128, 512], F32, tag="ps")
                    for i in range(4):
                        nc.tensor.matmul(oz[0:65, :], Vt[:, i, 65 * hh:65 * hh + 65],
                                         Eb[:, i, :], start=(i == 0), stop=(i == 3))
                    lnz = tpb.tile([128, 512], F32, tag="lnz")
                    nc.scalar.activation(out=lnz[64:65, :], in_=oz[64:65, :],
                                         func=AF.Ln)
                    rz = tpb.tile([128, 512], DT, tag="rz")
                    nc.scalar.activation(out=rz[64:65, :], in_=lnz[64:65, :],
                                         func=AF.Exp, scale=-1.0)
                    zb = ps.tile([128, 512], F32, tag="ps")
                    nc.tensor.matmul(zb, ones[64:65, 0:128], rz[64:65, :],
                                     start=True, stop=True)
                    zcp = otp.tile([128, 512], F32, tag="ot")
                    nc.vector.tensor_copy(out=zcp[0:64, :], in_=zb[0:64, :])
                    if hh % 2 == 0:
                        nc.vector.tensor_tensor(out=O_all[0:64, hh // 2, :],
                                                in0=oz[0:64, :], in1=zcp[0:64, :],
                                                op=ALU.mult)
                    else:
                        nc.vector.tensor_tensor(out=O_odd[0:64, hh // 2, :],
                                                in0=oz[0:64, :],
                                                in1=zcp[0:64, :], op=ALU.mult)
                if True:
                    nc.sync.dma_start(out=O_all[64:128, :, :], in_=O_odd[0:64, :, :])

                # ---- proj + residual ----
                wps = wpp.tile([128, 4, 512], DT, tag="wp")
                nc.sync.dma_start(out=wps, in_=pw[0 if DEV_STATIC_W else bi])
                x2 = xp.tile([128, 4, T], DT, tag="x")
                for f in range(4):
                    p_ps = ps.tile([128, 512], F32, tag="ps")
                    if not lean:
                        nc.tensor.matmul(p_ps, pbt[0:1, 128 * f:128 * f + 128],
                                         ones[0:1, :], start=True, stop=False)
                    for k in range(4):
                        nc.tensor.matmul(p_ps, wps[:, k, 128 * f:128 * f + 128],
                                         O_all[:, k, :], start=(lean and k == 0),
                                         stop=(k == 3))
                    nc.vector.tensor_tensor(out=x2[:, f, :], in0=x[:, f, :],
                                            in1=p_ps, op=ALU.add)
                x = x2

                # ---- FF ----
                xn2 = layer_norm(x, g2t, b2t, T)
                f2big = ps4.tile([128, 4, 512], F32, tag="ps4", name=f"f2big{bi}")
                f2ps = [f2big[:, i, :] for i in range(4)]
                if not lean:
                    for f in range(4):
                        nc.tensor.matmul(f2ps[f], f2bt[0:1, 128 * f:128 * f + 128],
                                         ones[0:1, :], start=True, stop=False)
                for q in range(4):
                    w1s = w1p.tile([128, 4, 512], DT, tag="w1")
                    nc.gpsimd.dma_start(out=w1s, in_=f1w[0 if DEV_STATIC_W else step, :, :, 512 * q:512 * q + 512])
                    hq = hp.tile([128, 4, T], DT, tag="h")
                    for ft in range(4):
                        h_ps = ps.tile([128, 512], F32, tag="ps")
                        for k in range(4):
                            nc.tensor.matmul(h_ps, w1s[:, k, 128 * ft:128 * ft + 128],
                                             xn2[:, k, :], start=(k == 0), stop=(k == 3))
                        fb = 4 * q + ft
                        nc.scalar.activation(out=hq[:, ft, :], in_=h_ps, func=AF.Gelu,
                                             bias=0.0 if lean else f1bt[:, fb:fb + 1])
                    w2s = w2p.tile([128, 4, 512], DT, tag="w2")
                    nc.gpsimd.dma_start(out=w2s, in_=f2w[0 if DEV_STATIC_W else step, :, 4 * q:4 * q + 4, :])
                    for f in range(4):
                        for kk in range(4):
                            nc.tensor.matmul(f2ps[f], w2s[:, kk, 128 * f:128 * f + 128],
                                             hq[:, kk, :],
                                             start=(lean and q == 0 and kk == 0),
                                             stop=(q == 3 and kk == 3))
                x3 = xp.tile([128, 4, T], DT, tag="x")
                for f in range(4):
                    nc.vector.tensor_tensor(out=x3[:, f, :], in0=x[:, f, :],
                                            in1=f2ps[f], op=ALU.add)
                x = x3

            # ---- head: lnf -> bf16 logits (token-partition) + logZ ----
            trunk_ctx.close()
            wop = ctx.enter_context(tc.tile_pool(name="wop", bufs=3))
            lsp = ctx.enter_context(tc.tile_pool(name="lsp", bufs=2))
            gft = bp.tile([128, 4], F32, tag="g1t")
            bft = bp.tile([128, 4], F32, tag="b1t")
            nc.sync.dma_start(out=gft, in_=gf[:, :])
            nc.sync.dma_start(out=bft, in_=bf[:, :])
            xnf = layer_norm(x, gft, bft, TL)
            xnb = constp.tile([128, 4, TL], BF)
            for k in range(4):
                nc.vector.tensor_copy(out=xnb[:, k, :], in_=xnf[:, k, 0:TL])

            bo = ctx.enter_context(tc.tile_pool(name="bop", bufs=1))
            bot = bo.tile([1, V], BF)
            nc.sync.dma_start(out=bot, in_=bout[0:1, :])
            GRP = 4                      # vocab chunks per DMA batch
            VB = GRP * VCH               # 2048
            ngrp = (V + VB - 1) // VB    # 16 (last partial: 32000 = 15*2048 + 1280)
            for tt in range(2):
                zacc = tpb.tile([128, 64], F32, tag="zacc")
                ci = 0
                for gi in range(ngrp):
                    g0 = gi * VB
                    gn = min(VB, V - g0)
                    wos = wop.tile([128, 4, VB], BF, tag="wo")
                    eng = nc.scalar if gi % 2 == 0 else nc.sync
                    eng.dma_start(out=wos[:, :, 0:gn], in_=wout[:, :, g0:g0 + gn])
                    lsb = lsp.tile([128, VB], F32, tag="lsb")
                    nsub = (gn + VCH - 1) // VCH
                    for si in range(nsub):
                        v0 = g0 + si * VCH
                        vn = min(VCH, V - v0)
                        l_ps = ps.tile([128, 512], F32, tag="ps")
                        if not lean:
                            nc.tensor.matmul(l_ps[:, 0:vn], ones_bf[0:1, :],
                                             bot[0:1, v0:v0 + vn], start=True, stop=False)
                        for k in range(4):
                            nc.tensor.matmul(l_ps[:, 0:vn],
                                             xnb[:, k, 128 * tt:128 * tt + 128],
                                             wos[:, k, si * VCH:si * VCH + vn],
                                             start=(lean and k == 0), stop=(k == 3))
                        nc.vector.tensor_copy(out=lsb[:, si * VCH:si * VCH + vn],
                                              in_=l_ps[:, 0:vn])
                        esc = lsp.tile([128, 512], F32, tag="esc")
                        nc.scalar.activation(out=esc[:, 0:vn], in_=l_ps[:, 0:vn],
                                             func=AF.Exp, accum_out=zacc[:, ci:ci + 1])
                        ci += 1
                    nc.gpsimd.dma_start(out=logits[128 * tt:128 * tt + 128, g0:g0 + gn],
                                        in_=lsb[:, 0:gn])
                z = tp.tile([128, 1], F32, tag="z")
                nc.vector.reduce_sum(z, zacc[:, 0:ci], axis=mybir.AxisListType.X)
                lz = tp.tile([128, 1], F32, tag="lz")
                nc.scalar.activation(out=lz, in_=z, func=AF.Ln)
                nc.sync.dma_start(out=logz[tt:tt + 1, :], in_=lz[:, 0:1])

    nc.compile()
    return nc


def _prep_inputs(inputs):
    """Host-side sharding: per-core in_maps (identical program, per-core data)."""
    f32 = np.float32
    bf16 = ml_dtypes.bfloat16
    idx = np.asarray(inputs["idx"])
    tok_emb = np.asarray(inputs["tok_emb"], f32)
    pos_emb = np.asarray(inputs["pos_emb"], f32)

    def dtile(a2d):
        # [D, cols] -> [128, 4, cols]  (d = k*128 + p)
        return np.ascontiguousarray(a2d.reshape(4, 128, -1).transpose(1, 0, 2))

    # weights, shared across cores
    shared = {}
    qkw = np.zeros((NBLK, 128, 4, 1024), f32)
    vw = np.zeros((NBLK, 128, 4, 512), f32)
    pw = np.zeros((NBLK, 128, 4, 512), f32)
    pblk = np.zeros((NBLK, 128, 40), f32)
    prow = np.zeros((NBLK, 1, 1536), f32)
    pack_ln1 = lambda a: np.asarray(a, f32).reshape(4, 128).T
    for l in range(L):
        for blk in range(2):
            bi = 2 * l + blk
            w = np.asarray(inputs["un_qkv_w" if blk == 0 else "m_qkv_w"][l], f32)
            b = np.asarray(inputs["un_qkv_b" if blk == 0 else "m_qkv_b"][l], f32)
            wq_ = np.concatenate([w[0], w[1]], axis=1)       # [D, 1024] q|k
            qkw[bi] = dtile(wq_)
            vw[bi] = dtile(w[2])
            pw_ = np.asarray(inputs["un_proj_w" if blk == 0 else "m_proj_w"][l], f32)
            pb_ = np.asarray(inputs["un_proj_b" if blk == 0 else "m_proj_b"][l], f32)
            pw[bi] = dtile(pw_)
            pblk[bi, :, 0:8] = np.concatenate([b[0], b[1]]).reshape(8, 128).T
            pblk[bi, :, 8:12] = pack_ln1(inputs["ln1_g"][l])
            pblk[bi, :, 12:16] = pack_ln1(inputs["ln1_b"][l])
            pblk[bi, :, 16:20] = pack_ln1(inputs["ln2_g"][l])
            pblk[bi, :, 20:24] = pack_ln1(inputs["ln2_b"][l])
            pblk[bi, :, 24:40] = np.asarray(inputs["ff_b1"][l], f32).reshape(16, 128).T
            prow[bi, 0, 0:512] = b[2]
            prow[bi, 0, 512:1024] = pb_
            prow[bi, 0, 1024:1536] = np.asarray(inputs["ff_b2"][l], f32)
    f1w = np.stack([dtile(np.asarray(inputs["ff_w1"][l], f32)) for l in range(L)])
    f2w = np.stack([np.ascontiguousarray(
        np.asarray(inputs["ff_w2"][l], f32).reshape(16, 128, 512).transpose(1, 0, 2))
        for l in range(L)])
    shared.update(qkw=qkw, vw=vw, pw=pw, f1w=f1w, f2w=f2w,
                  pblk=pblk, prow=prow,
                  gf=np.asarray(inputs["lnf_g"], f32).reshape(4, 128).T,
                  bf=np.asarray(inputs["lnf_b"], f32).reshape(4, 128).T,
                  wout=dtile(np.asarray(inputs["w_out"], f32)).astype(bf16),
                  bout=np.asarray(inputs["b_out"], f32).reshape(1, V).astype(bf16))

    in_maps = []
    for c in range(8):
        s, h = c // 2, c % 2
        perm = np.r_[h * TL:(h + 1) * TL, (1 - h) * TL:(2 - h) * TL]
        xe = dtile(tok_emb[idx[s, perm]].T)
        pe = dtile(pos_emb[perm].T)
        gl = perm  # glob token index per permuted position
        mask2d = (gl[:, None] <= gl[None, :]).astype(f32)      # [tk', tq'] allow
        mk = np.ascontiguousarray(mask2d.reshape(4, 128, T).transpose(1, 0, 2))
        m = dict(shared)
        m.update(xe=xe, pe=pe, msk=mk)
        in_maps.append(m)
    return in_maps


def kernel(**inputs):
    from concourse.bass_utils import run_bass_kernel_spmd

    lean = all(
        not np.any(np.asarray(inputs[k]))
        for k in ("un_qkv_b", "un_proj_b", "m_qkv_b", "m_proj_b",
                  "ff_b1", "ff_b2", "ln1_b", "ln2_b", "lnf_b", "b_out")
    ) and all(
        np.all(np.asarray(inputs[k]) == 1.0)
        for k in ("ln1_g", "ln2_g", "lnf_g")
    )
    key = ("nc", lean)
    if key not in _cache:
        _cache[key] = _build_nc(lean=lean)
        _cache["nc"] = _cache[key]
    nc = _cache[key]
    in_maps = _prep_inputs(inputs)
    res = run_bass_kernel_spmd(nc, in_maps, core_ids=list(range(8)))

    idx = np.asarray(inputs["idx"])
    targets = np.asarray(inputs["targets"])
    logits = np.zeros((B, T, V), np.float32)
    logz = np.zeros((B, T), np.float32)
    for c in range(8):
        s, h = c // 2, c % 2
        sl = slice(h * TL, (h + 1) * TL)
        logits[s, sl] = res.results[c]["logits"]
        logz[s, sl] = res.results[c]["logz"].reshape(TL)
    gathered = np.take_along_axis(
        logits.reshape(B * T, V),
        targets.reshape(B * T, 1).astype(np.int64), 1)[:, 0]
    loss = np.float32(np.mean(logz.reshape(B * T) - gathered))
    return logits, loss


# revision 24
# speedup vs baseline: 1.1093x; 1.0445x over previous
"""Trainium2 Bass kernel for nn_Enigma_30502857736394 (dense transformer fwd + loss).

Sharding (8 cores, zero collectives):
  core c -> sequence s = c//2, token-half h = c%2.
  Both cores of a pair redundantly compute the full-sequence (T=512) trunk;
  the vocab head is token-split: each core does its own 256 tokens x full V.
  Each core PERMUTES tokens so its half sits at columns 0:255 (the trunk is
  token-permutation-equivariant; mask/embeddings carry the permutation), so
  one identical SPMD program serves all 8 cores.

Layouts: activations feature-partition [d, t] (f32r); weights f32r streamed
from DRAM; attention computed as S^T = K^T-layout matmul, exp without max
subtraction (scores bounded: weights ~N(0, 0.02)), causal mask multiplied in
as data, softmax denominator via a ones-column appended to V^T, output
divided by broadcast Z. Head in bf16 (DMA-bound there), logits fp32 out.
Loss assembled on host from device logits + device logZ.
"""
import numpy as np
import ml_dtypes

B, T, D, H, L, V = 4, 512, 512, 8, 8, 32000
HS, DF, EPS = 64, 4 * 512, 1e-5
TL = 256          # tokens per core in the head
NBLK = 2 * L      # 16 transformer blocks
VCH = 512         # head vocab chunk

_cache = {}
DEV_STATIC_W = False   # reuse block-0 weights (kill per-block weight DMAs)
DEV_FAST_LN = False    # skip Ln/Exp chain in LN (wrong numerics, timing only)


def _build_nc(lean=True):
    # lean=True: biases all zero + LN affine identity (verified by caller)
    import concourse.bacc as bacc
    import concourse.mybir as mybir
    import concourse.tile as tile
    import concourse.bass as bass

    DT = mybir.dt.float32r
    F32 = mybir.dt.float32
    BF = mybir.dt.bfloat16
    AF = mybir.ActivationFunctionType
    ALU = mybir.AluOpType

    # Route Ln and Exp to the combined natural_log_exp set (otherwise the
    # table-load pass picks separate sets and thrashes 5 loads per block).
    from concourse.hw_specs import get_activation_tables
    tabs = get_activation_tables("gen3")
    if AF.Exp in tabs["exp_and_others"]:
        assert AF.Exp in tabs["natural_log_exp_and_others"]
        assert AF.Ln in tabs["natural_log_exp_and_others"]
        tabs["exp_and_others"].discard(AF.Exp)
        tabs["natural_log"].discard(AF.Ln)

    nc = bacc.Bacc(trn_type="TRN2")

    din = lambda n, s, dt=DT: nc.dram_tensor(n, s, dt, kind="ExternalInput")
    xe = din("xe", (128, 4, T), F32)
    pe = din("pe", (128, 4, T), F32)
    msk = din("msk", (128, 4, T))
    qkw = din("qkw", (NBLK, 128, 4, 1024))
    vw = din("vw", (NBLK, 128, 4, 512))
    pw = din("pw", (NBLK, 128, 4, 512))
    f1w = din("f1w", (L, 128, 4, DF))
    f2w = din("f2w", (L, 128, 16, 512))
    pblk = din("pblk", (NBLK, 128, 40), F32)   # qkb|g1|b1|g2|b2|f1b columns
    prow = din("prow", (NBLK, 1, 1536))        # vbr|pbr|f2br rows (f32r)
    gf = din("gf", (128, 4), F32)
    bf = din("bf", (128, 4), F32)
    wout = din("wout", (128, 4, V), BF)
    bout = din("bout", (1, V), BF)
    logits = nc.dram_tensor("logits", (TL, V), F32, kind="ExternalOutput")
    logz = nc.dram_tensor("logz", (2, 128), F32, kind="ExternalOutput")

    from contextlib import ExitStack
    with ExitStack() as ctx:
        tc = ctx.enter_context(tile.TileContext(nc))
        pool = lambda n, b, **kw: ctx.enter_context(tc.tile_pool(name=n, bufs=b, **kw))
        constp = pool("const", 1)
        xp = pool("xp", 2)
        xnp = pool("xnp", 1)
        sqp = pool("sqp", 2)
        otp = pool("otp", 2)
        bp = pool("bp", 2)
        tp = pool("tp", 1)
        tpb = pool("tpb", 2)
        ps = pool("ps", 4, space="PSUM")
        ps4 = pool("ps4", 2, space="PSUM")
        trunk_ctx = ExitStack()
        tpool = lambda n, b, **kw: trunk_ctx.enter_context(tc.tile_pool(name=n, bufs=b, **kw))
        yp = tpool("yp", 1)
        vtp = tpool("vtp", 1)
        ep = tpool("ep", 2)
        opl = tpool("op", 1)
        hp = tpool("hp", 2)
        wq = tpool("wq", 2)
        wv = tpool("wv", 1)
        wpp = tpool("wpp", 1)
        w1p = tpool("w1p", 2)
        w2p = tpool("w2p", 2)
        if True:
            ones_f = constp.tile([128, 512], F32)
            nc.vector.memset(ones_f, 1.0)
            ones = constp.tile([128, 512], DT)
            nc.vector.tensor_copy(out=ones, in_=ones_f)
            cvals = [0.0, 1.0, -1.0, EPS, -0.5, float(HS) ** -0.5]
            cbuf = constp.tile([128, len(cvals)], F32)
            for i, v in enumerate(cvals):
                nc.vector.memset(cbuf[:, i:i + 1], v)
                nc.const_aps.aps[(F32, float(v))] = cbuf[:, i:i + 1]
            ones_bf = constp.tile([1, 128], BF)
            nc.vector.tensor_copy(out=ones_bf, in_=ones_f[0:1, 0:128])
            msk_sb = constp.tile([128, 4, T], DT)
            nc.sync.dma_start(out=msk_sb, in_=msk[:, :, :])

            # x = xe + pe  (residual stream, f32r, [d-part, d-tile, t])
            x = xp.tile([128, 4, T], DT, tag="x")
            xet = hp.tile([128, 4, T], F32, tag="h")
            pet = hp.tile([128, 4, T], F32, tag="h")
            nc.sync.dma_start(out=xet, in_=xe[:, :, :])
            nc.sync.dma_start(out=pet, in_=pe[:, :, :])
            nc.vector.tensor_tensor(out=x[:, :, :], in0=xet, in1=pet, op=ALU.add)

            def layer_norm(x_t, g_t, b_t, tcols):
                xn_t = xnp.tile([128, 4, T], DT, tag="xn")
                s1 = ps.tile([128, 512], F32, tag="ps")
                s2 = ps.tile([128, 512], F32, tag="ps")
                for k in range(4):
                    sq = sqp.tile([128, 512], DT, tag="sq")
                    nc.vector.tensor_tensor(out=sq[:, 0:tcols], in0=x_t[:, k, 0:tcols],
                                            in1=x_t[:, k, 0:tcols], op=ALU.mult)
                    nc.tensor.matmul(s1[0:1, 0:tcols], ones[:, 0:1], x_t[:, k, 0:tcols],
                                     start=(k == 0), stop=(k == 3))
                    nc.tensor.matmul(s2[0:1, 0:tcols], ones[:, 0:1], sq[:, 0:tcols],
                                     start=(k == 0), stop=(k == 3))
                mu = tpb.tile([1, 512], DT, tag="mu")
                m2 = tp.tile([1, 512], F32, tag="m2")
                var = tp.tile([1, 512], F32, tag="var")
                rstd = tpb.tile([1, 512], DT, tag="rstd")
                nc.vector.tensor_scalar(out=mu[0:1, 0:tcols], in0=s1[0:1, 0:tcols],
                                        scalar1=1.0 / D, scalar2=None, op0=ALU.mult)
                nc.vector.tensor_scalar(out=m2[0:1, 0:tcols], in0=s2[0:1, 0:tcols],
                                        scalar1=1.0 / D, scalar2=None, op0=ALU.mult)
                t1 = tp.tile([1, 512], F32, tag="t1")
                nc.vector.tensor_tensor(out=t1[0:1, 0:tcols], in0=mu[0:1, 0:tcols],
                                        in1=mu[0:1, 0:tcols], op=ALU.mult)
                nc.vector.tensor_tensor(out=var[0:1, 0:tcols], in0=m2[0:1, 0:tcols],
                                        in1=t1[0:1, 0:tcols], op=ALU.subtract)
                if DEV_FAST_LN:
                    nc.vector.tensor_copy(out=rstd[0:1, 0:tcols], in_=var[0:1, 0:tcols])
                else:
                    lv = tp.tile([1, 512], F32, tag="lv")
                    nc.scalar.activation(out=lv[0:1, 0:tcols], in_=var[0:1, 0:tcols],
                                         func=AF.Ln, bias=EPS)
                    nc.scalar.activation(out=rstd[0:1, 0:tcols], in_=lv[0:1, 0:tcols],
                                         func=AF.Exp, scale=-0.5)
                mub = ps.tile([128, 512], F32, tag="ps")
                rsb = ps.tile([128, 512], F32, tag="ps")
                nc.tensor.matmul(mub[:, 0:tcols], ones[0:1, 0:128], mu[0:1, 0:tcols],
                                 start=True, stop=True)
                nc.tensor.matmul(rsb[:, 0:tcols], ones[0:1, 0:128], rstd[0:1, 0:tcols],
                                 start=True, stop=True)
                for k in range(4):
                    if lean:
                        sc = sqp.tile([128, 512], F32, tag="sq")
                        nc.vector.tensor_tensor(out=sc[:, 0:tcols],
                                                in0=x_t[:, k, 0:tcols],
                                                in1=mub[:, 0:tcols], op=ALU.subtract)
                        nc.vector.tensor_tensor(out=xn_t[:, k, 0:tcols],
                                                in0=sc[:, 0:tcols],
                                                in1=rsb[:, 0:tcols], op=ALU.mult)
                    else:
                        sc = sqp.tile([128, 512], F32, tag="sq")
                        nc.vector.tensor_tensor(out=sc[:, 0:tcols], in0=x_t[:, k, 0:tcols],
                                                in1=mub[:, 0:tcols], op=ALU.subtract)
                        nc.vector.tensor_tensor(out=sc[:, 0:tcols], in0=sc[:, 0:tcols],
                                                in1=rsb[:, 0:tcols], op=ALU.mult)
                        nc.vector.tensor_scalar(out=xn_t[:, k, 0:tcols], in0=sc[:, 0:tcols],
                                                scalar1=g_t[:, k:k + 1], scalar2=b_t[:, k:k + 1],
                                                op0=ALU.mult, op1=ALU.add)
                return xn_t

            for bi in range(NBLK):
                step = bi // 2
                masked = (bi % 2 == 1)
                pbt_all = bp.tile([128, 40], F32, tag="pbt_all")
                nc.gpsimd.dma_start(out=pbt_all, in_=pblk[bi])
                prt = tp.tile([1, 1536], DT, tag="prt")
                nc.gpsimd.dma_start(out=prt, in_=prow[bi])
                qkbt = pbt_all[:, 0:8]
                g1t = pbt_all[:, 8:12]
                b1t = pbt_all[:, 12:16]
                g2t = pbt_all[:, 16:20]
                b2t = pbt_all[:, 20:24]
                f1bt = pbt_all[:, 24:40]
                vbt = prt[0:1, 0:512]
                pbt = prt[0:1, 512:1024]
                f2bt = prt[0:1, 1024:1536]

                xn = layer_norm(x, g1t, b1t, T)

                # ---- qkv (q,k feature-partition; v transposed) ----
                Y = yp.tile([128, 8, T], DT, tag="Y")
                for half in range(2):
                    wqs = wq.tile([128, 4, 512], DT, tag="wq")
                    nc.sync.dma_start(out=wqs, in_=qkw[0 if DEV_STATIC_W else bi, :, :, 512 * half:512 * half + 512])
                    for ft in range(4):
                        f = 4 * half + ft
                        o_ps = ps.tile([128, 512], F32, tag="ps")
                        for k in range(4):
                            nc.tensor.matmul(o_ps, wqs[:, k, 128 * ft:128 * ft + 128],
                                             xn[:, k, :], start=(k == 0), stop=(k == 3))
                        if lean:
                            if f % 2 == 0:
                                nc.vector.tensor_copy(out=Y[:, f, :], in_=o_ps)
                            else:
                                nc.scalar.activation(out=Y[:, f, :], in_=o_ps,
                                                     func=AF.Copy)
                        else:
                            nc.vector.tensor_scalar(out=Y[:, f, :], in0=o_ps,
                                                    scalar1=qkbt[:, f:f + 1],
                                                    scalar2=None, op0=ALU.add)
                # V^T with ones column per head (Z fusion)
                wvs = wv.tile([128, 4, 512], DT, tag="wv")
                nc.sync.dma_start(out=wvs, in_=vw[0 if DEV_STATIC_W else bi])
                Vt = vtp.tile([128, 4, 520], DT, tag="Vt")
                for tt in range(4):
                    v_ps = ps.tile([128, 512], F32, tag="ps")
                    if not lean:
                        nc.tensor.matmul(v_ps, ones[0:1, 0:128], vbt[0:1, :],
                                         start=True, stop=False)
                    for k in range(4):
                        nc.tensor.matmul(v_ps, xn[:, k, 128 * tt:128 * tt + 128],
                                         wvs[:, k, :], start=(lean and k == 0),
                                         stop=(k == 3))
                    vt_h = Vt[:, tt, :].rearrange("p (h c) -> p h c", c=65)
                    nc.vector.tensor_copy(
                        out=vt_h[:, :, 0:64],
                        in_=v_ps.rearrange("p (h c) -> p h c", c=64))
                    nc.vector.tensor_copy(
                        out=vt_h[:, :, 64:65],
                        in_=ones_f[:, 0:8].rearrange("p (h c) -> p h c", c=1))

                # ---- attention per head ----
                O_all = opl.tile([128, 4, T], DT, tag="O")
                O_odd = hp.tile([128, 4, T], DT, tag="h")
                for hh in range(8):
                    bq = 64 * (hh % 2)
                    ftq, ftk = hh // 2, 4 + hh // 2
                    Ehalves = []
                    for j in range(2):
                        s_h = ps4.tile([128, 2, 512], F32, tag="ps4",
                                       name=f"S{hh}_{j}")
                        for i2 in range(2):
                            i = 2 * j + i2
                            nc.tensor.matmul(s_h[:, i2, :],
                                             Y[bq:bq + 64, ftk, 128 * i:128 * i + 128],
                                             Y[bq:bq + 64, ftq, :],
                                             start=True, stop=True)
                        Eh = ep.tile([128, 2, T], DT, tag=f"E{j}", name=f"E{hh}_{j}")
                        nc.scalar.activation(out=Eh[:, :, :], in_=s_h[:, :, :],
                                             func=AF.Exp, scale=float(HS) ** -0.5)
                        if masked:
                            meng = nc.vector if (hh + j) % 2 == 0 else nc.gpsimd
                            meng.tensor_tensor(out=Eh[:, :, :], in0=Eh[:, :, :],
                                               in1=msk_sb[:, 2 * j:2 * j + 2, :],
                                               op=ALU.mult)
                        Ehalves.append(Eh)
                    oz = ps.tile([128, 512], F32, tag="ps")
                    for i in range(4):
                        nc.tensor.matmul(oz[0:65, :], Vt[:, i, 65 * hh:65 * hh + 65],
                                         Ehalves[i // 2][:, i % 2, :],
                                         start=(i == 0), stop=(i == 3))
                    lnz = tpb.tile([128, 512], F32, tag="lnz")
                    nc.scalar.activation(out=lnz[64:65, :], in_=oz[64:65, :],
                                         func=AF.Ln)
                    rz = tpb.tile([128, 512], DT, tag="rz")
                    nc.scalar.activation(out=rz[64:65, :], in_=lnz[64:65, :],
                                         func=AF.Exp, scale=-1.0)
                    zb = ps.tile([128, 512], F32, tag="ps")
                    nc.tensor.matmul(zb, ones[64:65, 0:128], rz[64:65, :],
                                     start=True, stop=True)
                    zcp = otp.tile([128, 512], F32, tag="ot")
                    nc.vector.tensor_copy(out=zcp[0:64, :], in_=zb[0:64, :])
                    if hh % 2 == 0:
                        nc.vector.tensor_tensor(out=O_all[0:64, hh // 2, :],
                                                in0=oz[0:64, :], in1=zcp[0:64, :],
                                                op=ALU.mult)
                    else:
                        nc.vector.tensor_tensor(out=O_odd[0:64, hh // 2, :],
                                                in0=oz[0:64, :],
                                                in1=zcp[0:64, :], op=ALU.mult)
                if True:
                    nc.sync.dma_start(out=O_all[64:128, :, :], in_=O_odd[0:64, :, :])

                # ---- proj + residual ----
                wps = wpp.tile([128, 4, 512], DT, tag="wp")
                nc.sync.dma_start(out=wps, in_=pw[0 if DEV_STATIC_W else bi])
                x2 = xp.tile([128, 4, T], DT, tag="x")
                for f in range(4):
                    p_ps = ps.tile([128, 512], F32, tag="ps")
                    if not lean:
                        nc.tensor.matmul(p_ps, pbt[0:1, 128 * f:128 * f + 128],
                                         ones[0:1, :], start=True, stop=False)
                    for k in range(4):
                        nc.tensor.matmul(p_ps, wps[:, k, 128 * f:128 * f + 128],
                                         O_all[:, k, :], start=(lean and k == 0),
                                         stop=(k == 3))
                    nc.vector.tensor_tensor(out=x2[:, f, :], in0=x[:, f, :],
                                            in1=p_ps, op=ALU.add)
                x = x2

                # ---- FF ----
                xn2 = layer_norm(x, g2t, b2t, T)
                f2a = ps4.tile([128, 2, 512], F32, tag="ps4", name=f"f2a{bi}")
                f2b = ps4.tile([128, 2, 512], F32, tag="ps4", name=f"f2b{bi}")
                f2ps = [f2a[:, 0, :], f2a[:, 1, :], f2b[:, 0, :], f2b[:, 1, :]]
                if not lean:
                    for f in range(4):
                        nc.tensor.matmul(f2ps[f], f2bt[0:1, 128 * f:128 * f + 128],
                                         ones[0:1, :], start=True, stop=False)
                for q in range(4):
                    w1s = w1p.tile([128, 4, 512], DT, tag="w1")
                    nc.gpsimd.dma_start(out=w1s, in_=f1w[0 if DEV_STATIC_W else step, :, :, 512 * q:512 * q + 512])
                    hq = hp.tile([128, 4, T], DT, tag="h")
                    for ft in range(4):
                        h_ps = ps.tile([128, 512], F32, tag="ps")
                        for k in range(4):
                            nc.tensor.matmul(h_ps, w1s[:, k, 128 * ft:128 * ft + 128],
                                             xn2[:, k, :], start=(k == 0), stop=(k == 3))
                        fb = 4 * q + ft
                        nc.scalar.activation(out=hq[:, ft, :], in_=h_ps, func=AF.Gelu,
                                             bias=0.0 if lean else f1bt[:, fb:fb + 1])
                    w2s = w2p.tile([128, 4, 512], DT, tag="w2")
                    nc.gpsimd.dma_start(out=w2s, in_=f2w[0 if DEV_STATIC_W else step, :, 4 * q:4 * q + 4, :])
                    for f in range(4):
                        for kk in range(4):
                            nc.tensor.matmul(f2ps[f], w2s[:, kk, 128 * f:128 * f + 128],
                                             hq[:, kk, :],
                                             start=(lean and q == 0 and kk == 0),
                                             stop=(q == 3 and kk == 3))
                x3 = xp.tile([128, 4, T], DT, tag="x")
                for f in range(4):
                    nc.vector.tensor_tensor(out=x3[:, f, :], in0=x[:, f, :],
                                            in1=f2ps[f], op=ALU.add)
                x = x3

            # ---- head: lnf -> bf16 logits (token-partition) + logZ ----
            trunk_ctx.close()
            wop = ctx.enter_context(tc.tile_pool(name="wop", bufs=3))
            lsp = ctx.enter_context(tc.tile_pool(name="lsp", bufs=2))
            gft = bp.tile([128, 4], F32, tag="g1t")
            bft = bp.tile([128, 4], F32, tag="b1t")
            nc.sync.dma_start(out=gft, in_=gf[:, :])
            nc.sync.dma_start(out=bft, in_=bf[:, :])
            xnf = layer_norm(x, gft, bft, TL)
            xnb = constp.tile([128, 4, TL], BF)
            for k in range(4):
                nc.vector.tensor_copy(out=xnb[:, k, :], in_=xnf[:, k, 0:TL])

            bo = ctx.enter_context(tc.tile_pool(name="bop", bufs=1))
            bot = bo.tile([1, V], BF)
            nc.sync.dma_start(out=bot, in_=bout[0:1, :])
            GRP = 4                      # vocab chunks per DMA batch
            VB = GRP * VCH               # 2048
            ngrp = (V + VB - 1) // VB    # 16 (last partial: 32000 = 15*2048 + 1280)
            for tt in range(2):
                zacc = tpb.tile([128, 64], F32, tag="zacc")
                ci = 0
                for gi in range(ngrp):
                    g0 = gi * VB
                    gn = min(VB, V - g0)
                    wos = wop.tile([128, 4, VB], BF, tag="wo")
                    eng = nc.scalar if gi % 2 == 0 else nc.sync
                    eng.dma_start(out=wos[:, :, 0:gn], in_=wout[:, :, g0:g0 + gn])
                    lsb = lsp.tile([128, VB], F32, tag="lsb")
                    nsub = (gn + VCH - 1) // VCH
                    for si in range(nsub):
                        v0 = g0 + si * VCH
                        vn = min(VCH, V - v0)
                        l_ps = ps.tile([128, 512], F32, tag="ps")
                        if not lean:
                            nc.tensor.matmul(l_ps[:, 0:vn], ones_bf[0:1, :],
                                             bot[0:1, v0:v0 + vn], start=True, stop=False)
                        for k in range(4):
                            nc.tensor.matmul(l_ps[:, 0:vn],
                                             xnb[:, k, 128 * tt:128 * tt + 128],
                                             wos[:, k, si * VCH:si * VCH + vn],
                                             start=(lean and k == 0), stop=(k == 3))
                        nc.vector.tensor_copy(out=lsb[:, si * VCH:si * VCH + vn],
                                              in_=l_ps[:, 0:vn])
                        esc = lsp.tile([128, 512], F32, tag="esc")
                        nc.scalar.activation(out=esc[:, 0:vn], in_=l_ps[:, 0:vn],
                                             func=AF.Exp, accum_out=zacc[:, ci:ci + 1])
                        ci += 1
                    nc.gpsimd.dma_start(out=logits[128 * tt:128 * tt + 128, g0:g0 + gn],
                                        in_=lsb[:, 0:gn])
                z = tp.tile([128, 1], F32, tag="z")
                nc.vector.reduce_sum(z, zacc[:, 0:ci], axis=mybir.AxisListType.X)
                lz = tp.tile([128, 1], F32, tag="lz")
                nc.scalar.activation(out=lz, in_=z, func=AF.Ln)
                nc.sync.dma_start(out=logz[tt:tt + 1, :], in_=lz[:, 0:1])

    nc.compile()
    return nc


def _prep_inputs(inputs):
    """Host-side sharding: per-core in_maps (identical program, per-core data)."""
    f32 = np.float32
    bf16 = ml_dtypes.bfloat16
    idx = np.asarray(inputs["idx"])
    tok_emb = np.asarray(inputs["tok_emb"], f32)
    pos_emb = np.asarray(inputs["pos_emb"], f32)

    def dtile(a2d):
        # [D, cols] -> [128, 4, cols]  (d = k*128 + p)
        return np.ascontiguousarray(a2d.reshape(4, 128, -1).transpose(1, 0, 2))

    # weights, shared across cores
    shared = {}
    qkw = np.zeros((NBLK, 128, 4, 1024), f32)
    vw = np.zeros((NBLK, 128, 4, 512), f32)
    pw = np.zeros((NBLK, 128, 4, 512), f32)
    pblk = np.zeros((NBLK, 128, 40), f32)
    prow = np.zeros((NBLK, 1, 1536), f32)
    pack_ln1 = lambda a: np.asarray(a, f32).reshape(4, 128).T
    for l in range(L):
        for blk in range(2):
            bi = 2 * l + blk
            w = np.asarray(inputs["un_qkv_w" if blk == 0 else "m_qkv_w"][l], f32)
            b = np.asarray(inputs["un_qkv_b" if blk == 0 else "m_qkv_b"][l], f32)
            wq_ = np.concatenate([w[0], w[1]], axis=1)       # [D, 1024] q|k
            qkw[bi] = dtile(wq_)
            vw[bi] = dtile(w[2])
            pw_ = np.asarray(inputs["un_proj_w" if blk == 0 else "m_proj_w"][l], f32)
            pb_ = np.asarray(inputs["un_proj_b" if blk == 0 else "m_proj_b"][l], f32)
            pw[bi] = dtile(pw_)
            pblk[bi, :, 0:8] = np.concatenate([b[0], b[1]]).reshape(8, 128).T
            pblk[bi, :, 8:12] = pack_ln1(inputs["ln1_g"][l])
            pblk[bi, :, 12:16] = pack_ln1(inputs["ln1_b"][l])
            pblk[bi, :, 16:20] = pack_ln1(inputs["ln2_g"][l])
            pblk[bi, :, 20:24] = pack_ln1(inputs["ln2_b"][l])
            pblk[bi, :, 24:40] = np.asarray(inputs["ff_b1"][l], f32).reshape(16, 128).T
            prow[bi, 0, 0:512] = b[2]
            prow[bi, 0, 512:1024] = pb_
            prow[bi, 0, 1024:1536] = np.asarray(inputs["ff_b2"][l], f32)
    f1w = np.stack([dtile(np.asarray(inputs["ff_w1"][l], f32)) for l in range(L)])
    f2w = np.stack([np.ascontiguousarray(
        np.asarray(inputs["ff_w2"][l], f32).reshape(16, 128, 512).transpose(1, 0, 2))
        for l in range(L)])
    shared.update(qkw=qkw, vw=vw, pw=pw, f1w=f1w, f2w=f2w,
                  pblk=pblk, prow=prow,
                  gf=np.asarray(inputs["lnf_g"], f32).reshape(4, 128).T,
                  bf=np.asarray(inputs["lnf_b"], f32).reshape(4, 128).T,
                  wout=dtile(np.asarray(inputs["w_out"], f32)).astype(bf16),
                  bout=np.asarray(inputs["b_out"], f32).reshape(1, V).astype(bf16))

    in_maps = []
    for c in range(8):
        s, h = c // 2, c % 2
        perm = np.r_[h * TL:(h + 1) * TL, (1 - h) * TL:(2 - h) * TL]
        xe = dtile(tok_emb[idx[s, perm]].T)
        pe = dtile(pos_emb[perm].T)
        gl = perm  # glob token index per permuted position
        mask2d = (gl[:, None] <= gl[None, :]).astype(f32)      # [tk', tq'] allow
        mk = np.ascontiguousarray(mask2d.reshape(4, 128, T).transpose(1, 0, 2))
        m = dict(shared)
        m.update(xe=xe, pe=pe, msk=mk)
        in_maps.append(m)
    return in_maps


def kernel(**inputs):
    from concourse.bass_utils import run_bass_kernel_spmd

    lean = all(
        not np.any(np.asarray(inputs[k]))
        for k in ("un_qkv_b", "un_proj_b", "m_qkv_b", "m_proj_b",
                  "ff_b1", "ff_b2", "ln1_b", "ln2_b", "lnf_b", "b_out")
    ) and all(
        np.all(np.asarray(inputs[k]) == 1.0)
        for k in ("ln1_g", "ln2_g", "lnf_g")
    )
    key = ("nc", lean)
    if key not in _cache:
        _cache[key] = _build_nc(lean=lean)
        _cache["nc"] = _cache[key]
    nc = _cache[key]
    in_maps = _prep_inputs(inputs)
    res = run_bass_kernel_spmd(nc, in_maps, core_ids=list(range(8)))

    idx = np.asarray(inputs["idx"])
    targets = np.asarray(inputs["targets"])
    logits = np.zeros((B, T, V), np.float32)
    logz = np.zeros((B, T), np.float32)
    for c in range(8):
        s, h = c // 2, c % 2
        sl = slice(h * TL, (h + 1) * TL)
        logits[s, sl] = res.results[c]["logits"]
        logz[s, sl] = res.results[c]["logz"].reshape(TL)
    gathered = np.take_along_axis(
        logits.reshape(B * T, V),
        targets.reshape(B * T, 1).astype(np.int64), 1)[:, 0]
    loss = np.float32(np.mean(logz.reshape(B * T) - gathered))
    return logits, loss
